# revision 13
# baseline (speedup 1.0000x reference)
"""Associative-embedding (AE) loss on 8 TRN2 NeuronCores, data-parallel over batch.

Per batch image b:
  g[m,k,:]  = tags[b, idx[b,m,k], :]                       (gather, T=8)
  mean[m,:] = sum_k vf*g / max(cnt,1)
  pull      = (1/max(n,1)) * sum_m (1/max(cnt,1)) * sum_k vf * mean_t (g-mean)^2
  push      = 0.5/max(n(n-1),1) * sum_{i!=j valid} exp(-||mean_i-mean_j||^2)
  out[b]    = [push, pull]

Sharding: B=64 images split across 8 cores (8 each); all reductions are
batch-local; host concatenates per-core outputs.

Device strategy (v3):
- Tags are uploaded as one f16 [32768, 128] tensor: each 256B row is a
  block of 16 consecutive tag rows spanning all 8 images, so block ids fit
  dma_gather's int16 indices at its minimum 256B element size. f16 keeps
  the DVE select pipeline in 2-byte fast mode; the only precision loss is
  the one-time f16 rounding of tags (~5e-4 relative, vs 2e-2 tolerance),
  because each gathered row is selected (not accumulated) bit-exactly.
- Only VALID joints are gathered (~half). The ~240 persons of a core are
  bin-packed two-per-partition (heaviest with lightest) so each partition
  holds at most S occupied joint slots; descriptors = 128*S (~2176) vs the
  dense 4352. The gather runs as 4 chunked instructions so DVE compute and
  the Pool descriptor generation pipeline behind the DMA transfers.
- Per chunk: gm = blk * msk (one-hot picks each joint's 16B sub-row out of
  its 256B block), r = sum_sub gm — exact, since each (slot,t) group has
  at most one nonzero. Squares for the pull term run on ACT per chunk.
- Partition p holds person pair (A=lighter, packed first; B=heavier). A
  means finish after chunk 3 (cntA <= S/2 always fits), so A's
  mean/rneg/PE-transpose overlap chunk 4's DMA.
- Push: meanT [9,256] (8 tag dims + rneg row) via two PE transposes; Gram
  matmuls fold -0.5*r_j into PSUM via a constant 0.5 lhs row against the
  rneg rhs row; ee = exp(2G - r_i) with rneg as ACT bias; per-image sums
  via two f32r [128,8]x[128,257] matmuls whose 257th column carries the
  per-person pull terms; final per-image push sum is one fused
  tensor_tensor_reduce against a host-built selection matrix. f32r runs
  the PE at 1 cycle/row (vs 4 for f32) at free size >= 256.
- Host applies only input-metadata affine fixups ((raw-n)*iq, raw*ipn).

The program is specialized on S (padded max pair load) and cached; any
input works (worst case S=34 rebuilds once).
"""

import numpy as np

import concourse.bass as bass
import concourse.tile as tile
from concourse import bacc, mybir
from concourse.bass_utils import run_bass_kernel_spmd

B, N, T = 64, 65536, 8
M, K = 30, 17
NCORES = 8
BL = B // NCORES          # images per core
P = 128                   # partitions; 2 persons per partition (A/B banks)
NPER = 2 * P              # person slots (240 real + 16 empty)
RPB = 16                  # tag rows per 256B f16 block
NBLK = BL * N // RPB      # 32768 blocks, fits int16
F32 = mybir.dt.float32
F32R = mybir.dt.float32r
F16 = mybir.dt.float16
I16 = mybir.dt.int16
U8 = mybir.dt.uint8

ALU = mybir.AluOpType
AX = mybir.AxisListType
ACT = mybir.ActivationFunctionType

POOL_FRAC = 0.4
USE_F32R = True
USE_TTR = False   # InstTensorTensorReduce is not in the deployed DVE ucode

IDENT_B = 512             # [128,128] f32 eye for PE transpose
REG2_B = 1024             # partitions 0-7: wsel rows; partition 8: 0.5-const


def _chunk_sizes(S):
    base, rem = S // 4, S % 4
    cs = [base + (1 if i < rem else 0) for i in range(4)]
    return [c for c in cs if c > 0]


def build_nc(S):
    chunks = []
    c0 = 0
    for c in _chunk_sizes(S):
        chunks.append((c0, c0 + c))
        c0 += c
    nck = len(chunks)
    a_end = chunks[-1][0]  # A-joints all live in slots < a_end (cntA <= S/2)

    MSK_B = S * 64        # f16 [S,16,2] pair-expanded sub-row one-hot
    MAB_B = S * 4         # f16 [S,2] person-A/B slot masks
    o_mab = MSK_B
    o_cnt = o_mab + MAB_B
    o_icnt = o_cnt + 8
    o_icT = o_icnt + 8
    o_wA = o_icT + 8
    o_wB = o_wA + 32
    o_id = o_wB + 32
    o_r2 = o_id + IDENT_B
    PK2_B = o_r2 + REG2_B
    PK1_B = 16 * S

    nc = bacc.Bacc("TRN2", target_bir_lowering=False, debug=False,
                   num_devices=NCORES)
    tags_e = nc.declare_dram_parameter("tags", [NBLK, P], F16, isOutput=False)
    pk1_e = nc.declare_dram_parameter("pk1", [P, PK1_B], U8, isOutput=False)
    pk2_e = nc.declare_dram_parameter("pk2", [P, PK2_B], U8, isOutput=False)
    out_e = nc.declare_dram_parameter("out", [BL, 2], F32, isOutput=True)

    with tile.TileContext(nc) as tc:
        with (
            tc.tile_pool(name="sb", bufs=1) as sb,
            tc.tile_pool(name="ps", bufs=1, space="PSUM") as ps,
        ):
            pk1 = sb.tile([P, PK1_B], U8, tag="pk1")
            nc.sync.dma_start(pk1[:], pk1_e[:])
            pk2 = sb.tile([P, PK2_B], U8, tag="pk2")
            nc.scalar.dma_start(pk2[:], pk2_e[:])

            msk = pk2[:, 0:MSK_B].bitcast(F16)  # [P, S*16*2] paired
            mab = pk2[:, o_mab:o_cnt].bitcast(F16).rearrange("p (s a) -> p s a", a=2)
            cnt2 = pk2[:, o_cnt:o_cnt + 8].bitcast(F32)
            icnt2 = pk2[:, o_icnt:o_icnt + 8].bitcast(F32)
            icT2 = pk2[:, o_icT:o_icT + 8].bitcast(F32)
            wA = pk2[:, o_wA:o_wA + 32].bitcast(F32)
            wB = pk2[:, o_wB:o_wB + 32].bitcast(F32)
            ident = pk2[:, o_id:o_id + IDENT_B].bitcast(F32)
            reg2 = pk2[:, o_r2:o_r2 + REG2_B].bitcast(F32)

            blk = sb.tile([P, S, P], F16, tag="blk")
            gm = sb.tile([P, S, 16, 8], F16, tag="gm")
            r = sb.tile([P, S, 8], F16, tag="r")
            hA = sb.tile([P, S, 8], F32, tag="hA")
            hB = sb.tile([P, S, 8], F32, tag="hB")
            rsq = sb.tile([P, S, 8], F32, tag="rsq")
            q = sb.tile([P, S], F32, tag="q")
            qab = sb.tile([P, S, 2], F32, tag="qab")
            s1 = sb.tile([P, 2, 8], F32, tag="s1")
            mnr = sb.tile([P, 18], F32, tag="mnr")
            msq = sb.tile([P, 16], F32, tag="msq")
            s2 = sb.tile([P, 2], F32, tag="s2")
            crn = sb.tile([P, 2], F32, tag="crn")
            ee2 = sb.tile([P, 2, 258], F32R if USE_F32R else F32, tag="ee2")
            mel = sb.tile([9, 256], F32R if USE_F32R else F32, tag="mel")
            mer = sb.tile([9, 256], F32R if USE_F32R else F32, tag="mer")
            wab = sb.tile([P, 16], F32R if USE_F32R else F32, tag="wab")
            tv = sb.tile([8, 256], F32, tag="tv")
            sraw = sb.tile([8, 2], F32, tag="sraw")

            tp = ps.tile([9, 256], F32, tag="tp", bufs=1, space="PSUM")
            gA = ps.tile([P, 256], F32, tag="gA", bufs=1, space="PSUM")
            gB = ps.tile([P, 256], F32, tag="gB", bufs=1, space="PSUM")
            u = ps.tile([8, 258], F32, tag="u", bufs=1, space="PSUM")

            # lhs constant row (0.5) for the rank-1 -0.5*r_j fold; early,
            # off the critical path (ACT idles until chunk-1 squares).
            # 9 partitions from 0 (partition-8-only access is illegal);
            # rows 0:8 are garbage here and overwritten from tp later.
            nc.scalar.copy(out=mel[0:9, :], in_=reg2[0:9, :])
            # wA|wB staged through DVE so they are f32r-rounded for the PE
            nc.vector.tensor_copy(out=wab[:], in_=pk2[:, o_wA:o_wA + 64].bitcast(F32))

            # chunked gathers: desc-gen serializes on Pool, transfers pipeline
            for (a, b) in chunks:
                ni = (b - a) * P
                nc.gpsimd.dma_gather(
                    out_ap=blk[:, a:b, :],
                    in_ap=tags_e[:],
                    idxs_ap=pk1[:, 16 * a:16 * a + ni // 8].bitcast(I16),
                    num_idxs=ni,
                    num_idxs_reg=ni,
                    elem_size=P,
                )

            def side_finish(side, s_end):
                # person-bank slot sums -> mean, -||mean||^2, PE transpose
                h = hA if side == 0 else hB
                o = 9 * side
                nc.gpsimd.tensor_tensor(
                    out=h[:, 0:s_end, :],
                    in0=r[:, 0:s_end, :],
                    in1=mab[:, 0:s_end, side:side + 1].broadcast_to((P, s_end, 8)),
                    op=ALU.mult,
                )
                nc.vector.reduce_sum(
                    out=s1[:, side:side + 1, :].rearrange("p a t -> p (a t)"),
                    in_=h[:, 0:s_end, :].rearrange("p s t -> p t s"),
                    axis=AX.X,
                )
                nc.vector.tensor_tensor(
                    out=mnr[:, o:o + 8],
                    in0=s1[:, side:side + 1, :].rearrange("p a t -> p (a t)"),
                    in1=icnt2[:, side:side + 1].broadcast_to((P, 8)),
                    op=ALU.mult,
                )
                nc.vector.scalar_tensor_tensor(
                    out=msq[:, 8 * side:8 * side + 8],
                    in0=mnr[:, o:o + 8], scalar=-1.0, in1=mnr[:, o:o + 8],
                    op0=ALU.mult, op1=ALU.mult,
                )
                nc.vector.reduce_sum(
                    out=mnr[:, o + 8:o + 9],
                    in_=msq[:, 8 * side:8 * side + 8],
                    axis=AX.X,
                )
                nc.tensor.matmul(
                    out=tp[0:9, P * side:P * (side + 1)],
                    lhsT=mnr[:, o:o + 9], rhs=ident,
                    is_transpose=True, start=True, stop=True,
                )

            def sel_mult(engine, a, b):
                # gm = blk * msk over slots [a, b): su-flattened, t split
                # (4,2) so every operand's last dim is packed 2-wide f16 --
                # that turns on the DVE 2x_1p fast mode (0.5 cycles/elem).
                su0, su1 = a * 16, b * 16
                engine.tensor_tensor(
                    out=gm[:].rearrange("p s u t -> p (s u) t")[
                        :, su0:su1].rearrange("p su (f g) -> p su f g", g=2),
                    in0=blk[:].rearrange("p s (u t) -> p (s u) t", t=8)[
                        :, su0:su1].rearrange("p su (f g) -> p su f g", g=2),
                    in1=msk.rearrange("p (su g) -> p su g", g=2)[
                        :, su0:su1].unsqueeze(2).broadcast_to(
                            (P, (b - a) * 16, 4, 2)),
                    op=ALU.mult,
                )

            for c, (a, b) in enumerate(chunks):
                # Pool takes the leading POOL_FRAC of each chunk's slots (it
                # idles after descriptor generation); DVE takes the rest.
                npool = int((b - a) * POOL_FRAC)
                if npool:
                    sel_mult(nc.gpsimd, a, a + npool)
                sel_mult(nc.vector, a + npool, b)
                # r = sum over sub-rows: <=1 nonzero per (s,t), exact in f16
                with nc.allow_low_precision(reason="one-hot select, exact"):
                    nc.vector.reduce_sum(
                        out=r[:, a:b, :],
                        in_=gm[:, a:b].rearrange("p s u t -> p s t u"),
                        axis=AX.X,
                    )
                nc.scalar.activation(out=rsq[:, a:b, :], in_=r[:, a:b, :],
                                     func=ACT.Square)
                if c == nck - 2:
                    side_finish(0, a_end)  # A bank done: overlap last DMA
            side_finish(1, S)

            # pull: q[p,s] = ||r_s||^2, split by bank, per-person s2
            nc.vector.reduce_sum(out=q[:], in_=rsq[:], axis=AX.X)
            nc.gpsimd.tensor_tensor(
                out=qab[:],
                in0=q[:].unsqueeze(2).broadcast_to((P, S, 2)),
                in1=mab, op=ALU.mult,
            )
            nc.vector.reduce_sum(
                out=s2[:], in_=qab[:].rearrange("p s a -> p a s"), axis=AX.X,
            )
            rnegv = mnr[:].rearrange("p (a c) -> p a c", a=2)[:, :, 8:9]
            nc.gpsimd.tensor_tensor(out=crn[:].unsqueeze(2), in0=cnt2[:, 0:2].unsqueeze(2),
                                    in1=rnegv, op=ALU.mult)
            nc.gpsimd.tensor_tensor(out=crn[:], in0=s2[:], in1=crn[:],
                                    op=ALU.add)
            nc.vector.tensor_tensor(
                out=ee2[:, :, 256:258],
                in0=crn[:].unsqueeze(2).broadcast_to((P, 2, 2)),
                in1=icT2[:, 0:2].unsqueeze(2).broadcast_to((P, 2, 2)),
                op=ALU.mult)

            # meanT (+rneg row) to SBUF; rhs on DVE, lhs rows 0:8 on ACT
            nc.vector.tensor_copy(out=mer[:], in_=tp[:])
            nc.scalar.copy(out=mel[0:8, :], in_=tp[0:8, :])

            for side, g in ((0, gA), (1, gB)):
                nc.tensor.matmul(
                    out=g[:],
                    lhsT=mel[:, P * side:P * (side + 1)],
                    rhs=mer[:],
                    start=True, stop=True,
                )
                nc.scalar.activation(
                    out=ee2[:, side, 0:256],
                    in_=g[:], func=ACT.Exp,
                    scale=2.0, bias=mnr[:, 9 * side + 8:9 * side + 9],
                )
                nc.tensor.matmul(
                    out=u[:],
                    lhsT=wab[:, 8 * side:8 * side + 8],
                    rhs=ee2[:, side, :],
                    start=(side == 0), stop=(side == 1),
                )

            # per-image push raw = sum_j u[m,j]*wsel[m,j]; pull raw = u[m,256]
            if USE_TTR:
                nc.vector.tensor_tensor_reduce(
                    out=tv[:], in0=u[0:8, 0:256], in1=reg2[0:8, :],
                    scale=1.0, scalar=0.0, op0=ALU.mult, op1=ALU.add,
                    accum_out=sraw[:, 0:1],
                )
            else:
                nc.vector.tensor_tensor(out=tv[:], in0=u[0:8, 0:256],
                                        in1=reg2[0:8, :], op=ALU.mult)
                nc.vector.reduce_sum(out=sraw[:, 0:1], in_=tv[:], axis=AX.X)
            nc.vector.tensor_copy(out=sraw[:, 1:2], in_=u[0:8, 256:257])
            nc.sync.dma_start(out_e[:], sraw[:])

    nc.compile()
    return nc, a_end


_NC_CACHE = {}


def _get_nc(S):
    key = (S, USE_F32R, USE_TTR)
    if key not in _NC_CACHE:
        _NC_CACHE[key] = build_nc(S)
    return _NC_CACHE[key]


def _pack_core(idxc, vfc, S, a_end):
    """idxc: [BL, M, K] int64 clipped; vfc: [BL, M, K] bool. Returns pk1,
    pk2 and the (nv, fv) host fixups."""
    cnt = vfc.sum(axis=2).reshape(-1)               # [BL*M]
    order = np.argsort(-cnt, kind="stable")         # persons by cnt desc
    slot_person = np.full(NPER, -1, dtype=np.int64)
    slot_person[:BL * M] = order
    pers_img = np.arange(BL * M) // M

    msk = np.zeros((P, S, 16, 2), dtype=np.float16)
    mabm = np.zeros((P, S, 2), dtype=np.float16)
    cnt2 = np.zeros((P, 2), dtype=np.float32)
    wAB = np.zeros((2, P, 8), dtype=np.float32)
    wsel = np.zeros((8, 256), dtype=np.float32)
    blkid = np.zeros((P, S), dtype=np.int16)

    idxf = idxc.reshape(BL * M, K)
    vff = vfc.reshape(BL * M, K)
    for p in range(P):
        pers = [slot_person[NPER - 1 - p], slot_person[p]]  # [A(light), B(heavy)]
        s = 0
        for side, pe in enumerate(pers):
            if pe < 0:
                continue
            img = int(pers_img[pe])
            ks = np.nonzero(vff[pe])[0]
            c = len(ks)
            cnt2[p, side] = c
            if c > 0:
                wAB[side, p, img] = 1.0
                wsel[img, P * side + p] = 1.0
            g = img * N + idxf[pe, ks]                      # global rows
            for gi in g:
                blkid[p, s] = gi >> 4
                msk[p, s, gi & 15, :] = 1.0
                mabm[p, s, side] = 1.0
                s += 1
        assert s <= S
        assert mabm[p, a_end:, 0].sum() == 0, "A-joints must fit early chunks"

    # gather indices per chunk, wrapped [16, ni/16] replicated x8
    pk1 = np.zeros((P, 16 * S), dtype=np.uint8)
    c0 = 0
    for cs in _chunk_sizes(S):
        ni = cs * P
        vals = blkid[:, c0:c0 + cs].T.reshape(ni)           # item i = s*128+p
        wrapped = vals.reshape(ni // 16, 16).T
        pk1[:, 16 * c0:16 * (c0 + cs)] = np.tile(
            wrapped, (8, 1)).view(np.uint8).reshape(P, 2 * ni // 16)
        c0 += cs

    icnt2 = 1.0 / np.maximum(cnt2, 1.0)
    icT2 = (icnt2 / T).astype(np.float32)

    MSK_B = S * 64
    MAB_B = S * 4
    o_r2 = MSK_B + MAB_B + 24 + 64 + IDENT_B
    PK2_B = o_r2 + REG2_B
    pk2 = np.zeros((P, PK2_B), dtype=np.uint8)
    o = 0
    for arr in (msk, mabm, cnt2, icnt2, icT2, wAB[0], wAB[1]):
        bb = np.ascontiguousarray(arr).view(np.uint8).reshape(P, -1)
        pk2[:, o:o + bb.shape[1]] = bb
        o += bb.shape[1]
    pk2[:, o:o + IDENT_B] = np.eye(P, dtype=np.float32).view(np.uint8).reshape(P, IDENT_B)
    o += IDENT_B
    pk2[0:8, o:o + REG2_B] = wsel.view(np.uint8).reshape(8, REG2_B)
    pk2[8, o:o + REG2_B] = np.full(256, 0.5, dtype=np.float32).view(np.uint8)

    # host affine fixups
    cnt_im = vfc.sum(axis=2)
    n = (cnt_im > 0).sum(axis=1).astype(np.float32)         # [BL]
    iq = 0.5 * np.clip(n - 1.0, 0.0, 1.0) / np.maximum(n * (n - 1.0), 1.0)
    ipn = 1.0 / np.maximum(n, 1.0)
    nv = np.stack([n, np.zeros_like(n)], axis=1)
    fv = np.stack([iq, ipn], axis=1)
    return pk1, pk2, nv, fv


def make_in_maps(tags, keypoints, S, a_end):
    tags = np.asarray(tags, dtype=np.float32)
    kp = np.asarray(keypoints)
    idx = np.clip(kp[..., 0].astype(np.int64), 0, N - 1)
    vf = kp[..., 1] > 0

    in_maps, fixups = [], []
    for c in range(NCORES):
        sl = slice(BL * c, BL * (c + 1))
        pk1, pk2, nv, fv = _pack_core(idx[sl], vf[sl], S, a_end)
        in_maps.append({
            "tags": np.ascontiguousarray(
                tags[sl].reshape(NBLK, P).astype(np.float16)),
            "pk1": pk1,
            "pk2": pk2,
        })
        fixups.append((nv, fv))
    return in_maps, fixups


def _required_S(keypoints):
    vf = np.asarray(keypoints)[..., 1] > 0
    cnt = vf.sum(axis=2)                                    # [B, M]
    S = 4
    for c in range(NCORES):
        cc = np.sort(cnt[BL * c:BL * (c + 1)].reshape(-1))[::-1]
        cc = np.concatenate([cc, np.zeros(NPER - len(cc), dtype=cc.dtype)])
        S = max(S, int((cc[:P] + cc[NPER - 1:P - 1:-1]).max()))
    return S


def kernel(tags, keypoints):
    S = _required_S(keypoints)
    nc, a_end = _get_nc(S)
    in_maps, fixups = make_in_maps(tags, keypoints, S, a_end)
    last_err = None
    for _attempt in range(3):
        try:
            res = run_bass_kernel_spmd(nc, in_maps, core_ids=list(range(NCORES))).results
            break
        except Exception as e:  # a crashed predecessor can leave the NC wedged;
            last_err = e        # the failed attempt clears it, so retry
            import time
            time.sleep(1.0)
    else:
        raise last_err
    outs = []
    for c in range(NCORES):
        nv, fv = fixups[c]
        raw = res[c]["out"].reshape(BL, 2).astype(np.float32)
        outs.append((raw - nv) * fv)
    return np.concatenate(outs, axis=0).astype(np.float32)


# revision 15
# speedup vs baseline: 1.0707x; 1.0707x over previous
"""Associative-embedding (AE) loss on 8 TRN2 NeuronCores, data-parallel over batch.

Per batch image b:
  g[m,k,:]  = tags[b, idx[b,m,k], :]                       (gather, T=8)
  mean[m,:] = sum_k vf*g / max(cnt,1)
  pull      = (1/max(n,1)) * sum_m (1/max(cnt,1)) * sum_k vf * mean_t (g-mean)^2
  push      = 0.5/max(n(n-1),1) * sum_{i!=j valid} exp(-||mean_i-mean_j||^2)
  out[b]    = [push, pull]

Sharding: B=64 images split across 8 cores (8 each); all reductions are
batch-local; host concatenates per-core outputs.

Device strategy (v3):
- Tags are uploaded as one f16 [32768, 128] tensor: each 256B row is a
  block of 16 consecutive tag rows spanning all 8 images, so block ids fit
  dma_gather's int16 indices at its minimum 256B element size. f16 keeps
  the DVE select pipeline in 2-byte fast mode; the only precision loss is
  the one-time f16 rounding of tags (~5e-4 relative, vs 2e-2 tolerance),
  because each gathered row is selected (not accumulated) bit-exactly.
- Only VALID joints are gathered (~half). The ~240 persons of a core are
  bin-packed two-per-partition (heaviest with lightest) so each partition
  holds at most S occupied joint slots; descriptors = 128*S (~2176) vs the
  dense 4352. The gather runs as 4 chunked instructions so DVE compute and
  the Pool descriptor generation pipeline behind the DMA transfers.
- Per chunk: gm = blk * msk (one-hot picks each joint's 16B sub-row out of
  its 256B block), r = sum_sub gm — exact, since each (slot,t) group has
  at most one nonzero. Squares for the pull term run on ACT per chunk.
- Partition p holds person pair (A=lighter, packed first; B=heavier). A
  means finish after chunk 3 (cntA <= S/2 always fits), so A's
  mean/rneg/PE-transpose overlap chunk 4's DMA.
- Push: meanT [9,256] (8 tag dims + rneg row) via two PE transposes; Gram
  matmuls fold -0.5*r_j into PSUM via a constant 0.5 lhs row against the
  rneg rhs row; ee = exp(2G - r_i) with rneg as ACT bias; per-image sums
  via two f32r [128,8]x[128,257] matmuls whose 257th column carries the
  per-person pull terms; final per-image push sum is one fused
  tensor_tensor_reduce against a host-built selection matrix. f32r runs
  the PE at 1 cycle/row (vs 4 for f32) at free size >= 256.
- Host applies only input-metadata affine fixups ((raw-n)*iq, raw*ipn).

The program is specialized on S (padded max pair load) and cached; any
input works (worst case S=34 rebuilds once).
"""

import numpy as np

import concourse.bass as bass
import concourse.tile as tile
from concourse import bacc, mybir
from concourse.bass_utils import run_bass_kernel_spmd

B, N, T = 64, 65536, 8
M, K = 30, 17
NCORES = 8
BL = B // NCORES          # images per core
P = 128                   # partitions; 2 persons per partition (A/B banks)
NPER = 2 * P              # person slots (240 real + 16 empty)
RPB = 16                  # tag rows per 256B f16 block
NBLK = BL * N // RPB      # 32768 blocks, fits int16
F32 = mybir.dt.float32
F32R = mybir.dt.float32r
F16 = mybir.dt.float16
I16 = mybir.dt.int16
U8 = mybir.dt.uint8

ALU = mybir.AluOpType
AX = mybir.AxisListType
ACT = mybir.ActivationFunctionType

POOL_FRAC = 0.0   # cross-engine select split confuses the tile scheduler
USE_F32R = True
USE_TTR = False   # InstTensorTensorReduce is not in the deployed DVE ucode

IDENT_B = 512             # [128,128] f32 eye for PE transpose
REG2_B = 1024             # partitions 0-7: wsel rows; partition 8: 0.5-const


def _chunk_sizes(S):
    base, rem = S // 4, S % 4
    cs = [base + (1 if i < rem else 0) for i in range(4)]
    return [c for c in cs if c > 0]


def build_nc(S):
    chunks = []
    c0 = 0
    for c in _chunk_sizes(S):
        chunks.append((c0, c0 + c))
        c0 += c
    nck = len(chunks)
    a_end = chunks[-1][0]  # A-joints all live in slots < a_end (cntA <= S/2)

    MSK_B = S * 64        # f16 [S,16,2] pair-expanded sub-row one-hot
    MAB_B = S * 4         # f16 [S,2] person-A/B slot masks
    o_mab = MSK_B
    o_cnt = o_mab + MAB_B
    o_icnt = o_cnt + 8
    o_icT = o_icnt + 8
    o_wA = o_icT + 8
    o_wB = o_wA + 32
    o_id = o_wB + 32
    o_r2 = o_id + IDENT_B
    PK2_B = o_r2 + REG2_B
    PK1_B = 16 * S

    nc = bacc.Bacc("TRN2", target_bir_lowering=False, debug=False,
                   num_devices=NCORES)
    tags_e = nc.declare_dram_parameter("tags", [NBLK, P], F16, isOutput=False)
    pk1_e = nc.declare_dram_parameter("pk1", [P, PK1_B], U8, isOutput=False)
    pk2_e = nc.declare_dram_parameter("pk2", [P, PK2_B], U8, isOutput=False)
    out_e = nc.declare_dram_parameter("out", [BL, 2], F32, isOutput=True)

    with tile.TileContext(nc) as tc:
        with (
            tc.tile_pool(name="sb", bufs=1) as sb,
            tc.tile_pool(name="ps", bufs=1, space="PSUM") as ps,
        ):
            pk1 = sb.tile([P, PK1_B], U8, tag="pk1")
            nc.sync.dma_start(pk1[:], pk1_e[:])
            pk2 = sb.tile([P, PK2_B], U8, tag="pk2")
            nc.scalar.dma_start(pk2[:], pk2_e[:])

            msk = pk2[:, 0:MSK_B].bitcast(F16)  # [P, S*16*2] paired
            mab = pk2[:, o_mab:o_cnt].bitcast(F16).rearrange("p (s a) -> p s a", a=2)
            cnt2 = pk2[:, o_cnt:o_cnt + 8].bitcast(F32)
            icnt2 = pk2[:, o_icnt:o_icnt + 8].bitcast(F32)
            icT2 = pk2[:, o_icT:o_icT + 8].bitcast(F32)
            wA = pk2[:, o_wA:o_wA + 32].bitcast(F32)
            wB = pk2[:, o_wB:o_wB + 32].bitcast(F32)
            ident = pk2[:, o_id:o_id + IDENT_B].bitcast(F32)
            reg2 = pk2[:, o_r2:o_r2 + REG2_B].bitcast(F32)

            blk = sb.tile([P, S, P], F16, tag="blk")
            gm = sb.tile([P, S, 16, 8], F16, tag="gm")
            r = sb.tile([P, S, 8], F16, tag="r")
            hA = sb.tile([P, S, 8], F32, tag="hA")
            hB = sb.tile([P, S, 8], F32, tag="hB")
            rsq = sb.tile([P, S, 8], F32, tag="rsq")
            q = sb.tile([P, S], F32, tag="q")
            qab = sb.tile([P, S, 2], F32, tag="qab")
            s1 = sb.tile([P, 2, 8], F32, tag="s1")
            mnr = sb.tile([P, 18], F32R, tag="mnr")
            msq = sb.tile([P, 16], F32, tag="msq")
            s2 = sb.tile([P, 2], F32, tag="s2")
            crn = sb.tile([P, 2], F32, tag="crn")
            ee2 = sb.tile([P, 2, 258], F32R if USE_F32R else F32, tag="ee2")
            mel = sb.tile([9, 256], F32R if USE_F32R else F32, tag="mel")
            mer = sb.tile([9, 256], F32R if USE_F32R else F32, tag="mer")
            wab = sb.tile([P, 16], F32R if USE_F32R else F32, tag="wab")
            identr = sb.tile([P, P], F32R if USE_F32R else F32, tag="identr")
            tv = sb.tile([8, 256], F32, tag="tv")
            sraw = sb.tile([8, 2], F32, tag="sraw")

            tp = ps.tile([9, 256], F32R if USE_F32R else F32, tag="tp", bufs=1, space="PSUM")
            gA = ps.tile([P, 256], F32, tag="gA", bufs=1, space="PSUM")
            gB = ps.tile([P, 256], F32, tag="gB", bufs=1, space="PSUM")
            u = ps.tile([8, 258], F32, tag="u", bufs=1, space="PSUM")

            # lhs constant row (0.5) for the rank-1 -0.5*r_j fold; early,
            # off the critical path (ACT idles until chunk-1 squares).
            # 9 partitions from 0 (partition-8-only access is illegal);
            # rows 0:8 are garbage here and overwritten from tp later.
            nc.scalar.copy(out=mel[0:9, :], in_=reg2[0:9, :])
            # wA|wB staged through DVE so they are f32r-rounded for the PE
            nc.vector.tensor_copy(out=wab[:], in_=pk2[:, o_wA:o_wA + 64].bitcast(F32))
            nc.vector.tensor_copy(out=identr[:], in_=ident)

            # chunked gathers: desc-gen serializes on Pool, transfers pipeline
            for (a, b) in chunks:
                ni = (b - a) * P
                nc.gpsimd.dma_gather(
                    out_ap=blk[:, a:b, :],
                    in_ap=tags_e[:],
                    idxs_ap=pk1[:, 16 * a:16 * a + ni // 8].bitcast(I16),
                    num_idxs=ni,
                    num_idxs_reg=ni,
                    elem_size=P,
                )

            def side_finish(side, s_end):
                # person-bank slot sums -> mean, -||mean||^2, PE transpose
                h = hA if side == 0 else hB
                o = 9 * side
                nc.gpsimd.tensor_tensor(
                    out=h[:, 0:s_end, :],
                    in0=r[:, 0:s_end, :],
                    in1=mab[:, 0:s_end, side:side + 1].broadcast_to((P, s_end, 8)),
                    op=ALU.mult,
                )
                nc.vector.reduce_sum(
                    out=s1[:, side:side + 1, :].rearrange("p a t -> p (a t)"),
                    in_=h[:, 0:s_end, :].rearrange("p s t -> p t s"),
                    axis=AX.X,
                )
                nc.vector.tensor_tensor(
                    out=mnr[:, o:o + 8],
                    in0=s1[:, side:side + 1, :].rearrange("p a t -> p (a t)"),
                    in1=icnt2[:, side:side + 1].broadcast_to((P, 8)),
                    op=ALU.mult,
                )
                nc.vector.scalar_tensor_tensor(
                    out=msq[:, 8 * side:8 * side + 8],
                    in0=mnr[:, o:o + 8], scalar=-1.0, in1=mnr[:, o:o + 8],
                    op0=ALU.mult, op1=ALU.mult,
                )
                with nc.allow_low_precision(reason="f32r is f32-width"):
                    nc.vector.reduce_sum(
                        out=mnr[:, o + 8:o + 9],
                        in_=msq[:, 8 * side:8 * side + 8],
                        axis=AX.X,
                    )
                nc.tensor.matmul(
                    out=tp[0:9, P * side:P * (side + 1)],
                    lhsT=mnr[:, o:o + 9], rhs=identr[:],
                    is_transpose=True, start=True, stop=True,
                )

            def sel_mult(engine, a, b):
                # gm = blk * msk over slots [a, b): su-flattened, t split
                # (4,2) so every operand's last dim is packed 2-wide f16 --
                # that turns on the DVE 2x_1p fast mode (0.5 cycles/elem).
                su0, su1 = a * 16, b * 16
                engine.tensor_tensor(
                    out=gm[:].rearrange("p s u t -> p (s u) t")[
                        :, su0:su1].rearrange("p su (f g) -> p su f g", g=2),
                    in0=blk[:].rearrange("p s (u t) -> p (s u) t", t=8)[
                        :, su0:su1].rearrange("p su (f g) -> p su f g", g=2),
                    in1=msk.rearrange("p (su g) -> p su g", g=2)[
                        :, su0:su1].unsqueeze(2).broadcast_to(
                            (P, (b - a) * 16, 4, 2)),
                    op=ALU.mult,
                )

            for c, (a, b) in enumerate(chunks):
                # Pool takes the leading POOL_FRAC of each chunk's slots (it
                # idles after descriptor generation); DVE takes the rest.
                npool = int((b - a) * POOL_FRAC)
                if npool:
                    sel_mult(nc.gpsimd, a, a + npool)
                sel_mult(nc.vector, a + npool, b)
                # r = sum over sub-rows: <=1 nonzero per (s,t), exact in f16
                with nc.allow_low_precision(reason="one-hot select, exact"):
                    nc.vector.reduce_sum(
                        out=r[:, a:b, :],
                        in_=gm[:, a:b].rearrange("p s u t -> p s t u"),
                        axis=AX.X,
                    )
                nc.scalar.activation(out=rsq[:, a:b, :], in_=r[:, a:b, :],
                                     func=ACT.Square)
                if c == nck - 2:
                    side_finish(0, a_end)  # A bank done: overlap last DMA
            side_finish(1, S)

            # pull: q[p,s] = ||r_s||^2, split by bank, per-person s2
            nc.vector.reduce_sum(out=q[:], in_=rsq[:], axis=AX.X)
            nc.gpsimd.tensor_tensor(
                out=qab[:],
                in0=q[:].unsqueeze(2).broadcast_to((P, S, 2)),
                in1=mab, op=ALU.mult,
            )
            nc.vector.reduce_sum(
                out=s2[:], in_=qab[:].rearrange("p s a -> p a s"), axis=AX.X,
            )
            rnegv = mnr[:].bitcast(F32).rearrange("p (a c) -> p a c", a=2)[:, :, 8:9]
            nc.gpsimd.tensor_tensor(out=crn[:].unsqueeze(2), in0=cnt2[:, 0:2].unsqueeze(2),
                                    in1=rnegv, op=ALU.mult)
            nc.gpsimd.tensor_tensor(out=crn[:], in0=s2[:], in1=crn[:],
                                    op=ALU.add)
            nc.vector.tensor_tensor(
                out=ee2[:, :, 256:258],
                in0=crn[:].unsqueeze(2).broadcast_to((P, 2, 2)),
                in1=icT2[:, 0:2].unsqueeze(2).broadcast_to((P, 2, 2)),
                op=ALU.mult)

            # meanT (+rneg row) to SBUF; rhs on DVE, lhs rows 0:8 on ACT
            nc.vector.tensor_copy(out=mer[:], in_=tp[:])
            nc.scalar.copy(out=mel[0:8, :], in_=tp[0:8, :])

            for side, g in ((0, gA), (1, gB)):
                nc.tensor.matmul(
                    out=g[:],
                    lhsT=mel[:, P * side:P * (side + 1)],
                    rhs=mer[:],
                    start=True, stop=True,
                )
                nc.scalar.activation(
                    out=ee2[:, side, 0:256],
                    in_=g[:], func=ACT.Exp,
                    scale=2.0, bias=mnr[:, 9 * side + 8:9 * side + 9].bitcast(F32),
                )
                nc.tensor.matmul(
                    out=u[:],
                    lhsT=wab[:, 8 * side:8 * side + 8],
                    rhs=ee2[:, side, :],
                    start=(side == 0), stop=(side == 1),
                )

            # per-image push raw = sum_j u[m,j]*wsel[m,j]; pull raw = u[m,256]
            if USE_TTR:
                nc.vector.tensor_tensor_reduce(
                    out=tv[:], in0=u[0:8, 0:256], in1=reg2[0:8, :],
                    scale=1.0, scalar=0.0, op0=ALU.mult, op1=ALU.add,
                    accum_out=sraw[:, 0:1],
                )
            else:
                nc.vector.tensor_tensor(out=tv[:], in0=u[0:8, 0:256],
                                        in1=reg2[0:8, :], op=ALU.mult)
                nc.vector.reduce_sum(out=sraw[:, 0:1], in_=tv[:], axis=AX.X)
            nc.vector.tensor_copy(out=sraw[:, 1:2], in_=u[0:8, 256:257])
            nc.sync.dma_start(out_e[:], sraw[:])

    nc.compile()
    return nc, a_end


_NC_CACHE = {}


def _get_nc(S):
    key = (S, USE_F32R, USE_TTR)
    if key not in _NC_CACHE:
        _NC_CACHE[key] = build_nc(S)
    return _NC_CACHE[key]


def _pack_core(idxc, vfc, S, a_end):
    """idxc: [BL, M, K] int64 clipped; vfc: [BL, M, K] bool. Returns pk1,
    pk2 and the (nv, fv) host fixups."""
    cnt = vfc.sum(axis=2).reshape(-1)               # [BL*M]
    order = np.argsort(-cnt, kind="stable")         # persons by cnt desc
    slot_person = np.full(NPER, -1, dtype=np.int64)
    slot_person[:BL * M] = order
    pers_img = np.arange(BL * M) // M

    msk = np.zeros((P, S, 16, 2), dtype=np.float16)
    mabm = np.zeros((P, S, 2), dtype=np.float16)
    cnt2 = np.zeros((P, 2), dtype=np.float32)
    wAB = np.zeros((2, P, 8), dtype=np.float32)
    wsel = np.zeros((8, 256), dtype=np.float32)
    blkid = np.zeros((P, S), dtype=np.int16)

    idxf = idxc.reshape(BL * M, K)
    vff = vfc.reshape(BL * M, K)
    for p in range(P):
        pers = [slot_person[NPER - 1 - p], slot_person[p]]  # [A(light), B(heavy)]
        s = 0
        for side, pe in enumerate(pers):
            if pe < 0:
                continue
            img = int(pers_img[pe])
            ks = np.nonzero(vff[pe])[0]
            c = len(ks)
            cnt2[p, side] = c
            if c > 0:
                wAB[side, p, img] = 1.0
                wsel[img, P * side + p] = 1.0
            g = img * N + idxf[pe, ks]                      # global rows
            for gi in g:
                blkid[p, s] = gi >> 4
                msk[p, s, gi & 15, :] = 1.0
                mabm[p, s, side] = 1.0
                s += 1
        assert s <= S
        assert mabm[p, a_end:, 0].sum() == 0, "A-joints must fit early chunks"

    # gather indices per chunk, wrapped [16, ni/16] replicated x8
    pk1 = np.zeros((P, 16 * S), dtype=np.uint8)
    c0 = 0
    for cs in _chunk_sizes(S):
        ni = cs * P
        vals = blkid[:, c0:c0 + cs].T.reshape(ni)           # item i = s*128+p
        wrapped = vals.reshape(ni // 16, 16).T
        pk1[:, 16 * c0:16 * (c0 + cs)] = np.tile(
            wrapped, (8, 1)).view(np.uint8).reshape(P, 2 * ni // 16)
        c0 += cs

    icnt2 = 1.0 / np.maximum(cnt2, 1.0)
    icT2 = (icnt2 / T).astype(np.float32)

    MSK_B = S * 64
    MAB_B = S * 4
    o_r2 = MSK_B + MAB_B + 24 + 64 + IDENT_B
    PK2_B = o_r2 + REG2_B
    pk2 = np.zeros((P, PK2_B), dtype=np.uint8)
    o = 0
    for arr in (msk, mabm, cnt2, icnt2, icT2, wAB[0], wAB[1]):
        bb = np.ascontiguousarray(arr).view(np.uint8).reshape(P, -1)
        pk2[:, o:o + bb.shape[1]] = bb
        o += bb.shape[1]
    pk2[:, o:o + IDENT_B] = np.eye(P, dtype=np.float32).view(np.uint8).reshape(P, IDENT_B)
    o += IDENT_B
    pk2[0:8, o:o + REG2_B] = wsel.view(np.uint8).reshape(8, REG2_B)
    pk2[8, o:o + REG2_B] = np.full(256, 0.5, dtype=np.float32).view(np.uint8)

    # host affine fixups
    cnt_im = vfc.sum(axis=2)
    n = (cnt_im > 0).sum(axis=1).astype(np.float32)         # [BL]
    iq = 0.5 * np.clip(n - 1.0, 0.0, 1.0) / np.maximum(n * (n - 1.0), 1.0)
    ipn = 1.0 / np.maximum(n, 1.0)
    nv = np.stack([n, np.zeros_like(n)], axis=1)
    fv = np.stack([iq, ipn], axis=1)
    return pk1, pk2, nv, fv


def make_in_maps(tags, keypoints, S, a_end):
    tags = np.asarray(tags, dtype=np.float32)
    kp = np.asarray(keypoints)
    idx = np.clip(kp[..., 0].astype(np.int64), 0, N - 1)
    vf = kp[..., 1] > 0

    in_maps, fixups = [], []
    for c in range(NCORES):
        sl = slice(BL * c, BL * (c + 1))
        pk1, pk2, nv, fv = _pack_core(idx[sl], vf[sl], S, a_end)
        in_maps.append({
            "tags": np.ascontiguousarray(
                tags[sl].reshape(NBLK, P).astype(np.float16)),
            "pk1": pk1,
            "pk2": pk2,
        })
        fixups.append((nv, fv))
    return in_maps, fixups


def _required_S(keypoints):
    vf = np.asarray(keypoints)[..., 1] > 0
    cnt = vf.sum(axis=2)                                    # [B, M]
    S = 4
    for c in range(NCORES):
        cc = np.sort(cnt[BL * c:BL * (c + 1)].reshape(-1))[::-1]
        cc = np.concatenate([cc, np.zeros(NPER - len(cc), dtype=cc.dtype)])
        S = max(S, int((cc[:P] + cc[NPER - 1:P - 1:-1]).max()))
    return S


def kernel(tags, keypoints):
    S = _required_S(keypoints)
    nc, a_end = _get_nc(S)
    in_maps, fixups = make_in_maps(tags, keypoints, S, a_end)
    last_err = None
    for _attempt in range(3):
        try:
            res = run_bass_kernel_spmd(nc, in_maps, core_ids=list(range(NCORES))).results
            break
        except Exception as e:  # a crashed predecessor can leave the NC wedged;
            last_err = e        # the failed attempt clears it, so retry
            import time
            time.sleep(1.0)
    else:
        raise last_err
    outs = []
    for c in range(NCORES):
        nv, fv = fixups[c]
        raw = res[c]["out"].reshape(BL, 2).astype(np.float32)
        outs.append((raw - nv) * fv)
    return np.concatenate(outs, axis=0).astype(np.float32)
